# revision 10
# baseline (speedup 1.0000x reference)
"""Trainium2 Bass kernel for CTRLightGCN-style GNN message passing block.

Reference computation (per full input):
    A_g = row_normalized(A.sum(0)) + A_group                    # (4,25,25)
    xg = x.reshape(B, 4, 64, T, V)
    y  = einsum('gdc,gvw,bgctw->bgdtv', conv_w, A_g, xg).reshape(B, C, T, V)
    out = x + BN_train(y) * gamma + beta        (BN stats over B,T,V per C)

Strategy: data-parallel over batch B=64 across 8 cores (8 per core).

v2 design (vs the two-pass baseline):
  - x is loaded from HBM ONCE as fp16 (no pad) into 16 resident SBUF tiles
    [128, 3200]; output is written back as fp16 and upcast on host.  HBM
    traffic drops 53MB -> 26MB per core.
  - PE chain unchanged: MM1 (x chunk stationary, wblk streamed -> y1T in
    PSUM), MM2 (y1T stationary col-tiled by group pair, kron(I5,A^T)
    streamed -> y in PSUM).
  - Elementwise work is spread across engines (baseline was DVE-bound at
    194us busy):
      p1 evac (PSUM->y1t fp16)  -> Scalar engine
      p2 evac (PSUM->y16 fp16)  -> Vector tensor_scalar, with accum_out
                                   giving per-record channel SUMS for free
      sumsq                     -> Vector tensor_tensor_reduce on fp16 y16
                                   (2x rate), per 1600-col half slice
      BN affine (y16 in place)  -> GpSimd tensor_scalar
      +x residual               -> GpSimd DMA with accum_op=add
                                   (SBUF->SBUF, rides the DMA fabric)
  - Tiny (128,2) AllReduce of [sum, sumsq] per channel half; h=0's AR and
    pass 2 overlap h=1's pass 1.
"""
import numpy as np

import concourse.bacc as bacc
import concourse.tile as tile
from concourse import mybir
from concourse.bass_utils import run_bass_kernel_spmd

# ---- problem constants (hardcoded per contract) ----
B, C, T, V = 64, 256, 128, 25
G = 4
N_CORES = 8
B_LOC = B // N_CORES          # 8
TW = T * V                    # 3200
BN_EPS = 1e-5
N_PER_CH = B * TW             # 204800 (global per-channel count)

# chunk = 5 t-rows = 125 cols (last chunk 3 t = 75); batches of 4 chunks -> <=500 cols
CHUNK_M = [125] * 25 + [75]
BATCHES = []                  # list of (f0, [m...]) per (b,h)
_f = 0
_i = 0
while _i < len(CHUNK_M):
    ms = CHUNK_M[_i:_i + 4]
    if sum(ms) > 500:
        ms = CHUNK_M[_i:_i + 2]
    BATCHES.append((_f, ms))
    _f += sum(ms)
    _i += len(ms)
N_BAT = len(BATCHES)          # 7 (6x500 + 1x200)

F32 = mybir.dt.float32
F16 = mybir.dt.float16

# ---- engine assignment knobs ----
# p2-evac engine per global batch index (0..111): mostly vector, some scalar
EV2_SCALAR_EVERY = 9          # every 9th p2-evac goes to scalar (ACT)
N_WARM = 100                  # PE HAM warmup matmuls
W4_GPSIMD = False              # BN affine on GpSimd (else Vector)
W5_DMA_ACCUM = False           # +x residual via SWDGE DMA accum (else Vector)

_cache = {}


def _build():
    nc = bacc.Bacc()
    x16_in = nc.dram_tensor("x16", [B_LOC, 2, 128, TW], F16, kind="ExternalInput")
    wblk_in = nc.dram_tensor("wblk", [2, 128, 128], F16, kind="ExternalInput")
    arhs_in = nc.dram_tensor("arhs", [G, 125, 125], F16, kind="ExternalInput")
    gbn_in = nc.dram_tensor("gbn", [2, 128, 2], F32, kind="ExternalInput")
    out_d = nc.dram_tensor("out", [B_LOC, C, TW], F16, kind="ExternalOutput")

    with tile.TileContext(nc) as tc:
        with (
            tc.tile_pool(name="consts", bufs=1) as consts,
            tc.tile_pool(name="resid", bufs=1) as resid,
            tc.tile_pool(name="y1t", bufs=2) as y1tp,
            tc.tile_pool(name="ps1", bufs=3, space="PSUM") as ps1,
            tc.tile_pool(name="ps2", bufs=3, space="PSUM") as ps2,
            tc.tile_pool(name="psw", bufs=1, space="PSUM") as psw,
            tc.tile_pool(name="dr", bufs=1, space="DRAM") as dr,
        ):
            # ---- PE HAM warmup ----
            wtile = consts.tile([128, 128], F16, tag="warm")
            nc.vector.memset(wtile, 0.0)
            wp = psw.tile([128, 128], F32, tag="warmp")
            for _ in range(N_WARM):
                nc.tensor.matmul(wp, wtile, wtile, start=True, stop=True)
            wsink = consts.tile([128, 1], F32, tag="wsink")
            nc.scalar.copy(out=wsink, in_=wp[:, 0:1])

            # ---- constants ----
            wblk_t = []
            gbn_t = []
            arhs_t = []
            for h in range(2):
                w = consts.tile([128, 128], F16, tag=f"wblk{h}")
                nc.sync.dma_start(out=w, in_=wblk_in[h])
                wblk_t.append(w)
                gbt = consts.tile([128, 2], F32, tag=f"gbn{h}")
                nc.sync.dma_start(out=gbt, in_=gbn_in[h])
                gbn_t.append(gbt)
            for g in range(G):
                a = consts.tile([125, 125], F16, tag=f"arhs{g}")
                nc.sync.dma_start(out=a, in_=arhs_in[g])
                arhs_t.append(a)

            # ---- resident x tiles; all input DMAs issued upfront ----
            xr = []
            for h in range(2):
                for b in range(B_LOC):
                    xt = resid.tile([128, TW], F16, tag=f"xr{h}_{b}",
                                    name=f"xr{h}_{b}")
                    nc.sync.dma_start(out=xt, in_=x16_in[b, h])
                    xr.append(xt)

            y16 = [resid.tile([128, B_LOC, TW], F16, tag=f"y16_{h}",
                              name=f"y16_{h}")
                   for h in range(2)]
            # per-record channel sums (accum_out of p2 evac): 56 records/half
            sums_t = [consts.tile([128, B_LOC * N_BAT], F32, tag=f"sums{h}",
                                  name=f"sums{h}") for h in range(2)]
            # per-(b, quarter-slice) sumsq: 32 slots/half
            sq_t = [consts.tile([128, B_LOC * 4], F32, tag=f"sq{h}",
                                name=f"sq{h}") for h in range(2)]
            sqscr = resid.tile([128, 800], F16, tag="sqscr", name="sqscr")

            cc_in = [dr.tile([128, 2], F32, name=f"cci{h}") for h in range(2)]
            cc_out = [dr.tile([128, 2], F32, addr_space="Shared", name=f"cco{h}")
                      for h in range(2)]

            eps_t = consts.tile([128, 1], F32, tag="eps")
            nc.vector.memset(eps_t, BN_EPS)

            gidx = 0  # global batch counter for engine rotation

            def pass1_half(h):
                nonlocal gidx
                for b in range(B_LOC):
                    xt = xr[h * B_LOC + b]
                    for bi, (f0, ms) in enumerate(BATCHES):
                        used = sum(ms)
                        nch = len(ms)
                        p1 = ps1.tile([128, 4, 128], F32, tag="p1")
                        co = f0
                        for ci, m in enumerate(ms):
                            cols = min(128, TW - co)
                            nc.tensor.matmul(
                                p1[:cols, ci, :], xt[:, co:co + cols],
                                wblk_t[h], start=True, stop=True,
                            )
                            co += m
                        y1 = y1tp.tile([128, 4, 128], F16, tag="y1t")
                        nc.scalar.copy(out=y1[:, :nch, :], in_=p1[:, :nch, :])
                        p2 = ps2.tile([128, 500], F32, tag="p2")
                        co2 = 0
                        for ci, m in enumerate(ms):
                            for gl in range(2):
                                nc.tensor.matmul(
                                    p2[gl * 64:(gl + 1) * 64, co2:co2 + m],
                                    y1[0:m, ci, gl * 64:(gl + 1) * 64],
                                    arhs_t[2 * h + gl][:m, :m],
                                    start=True, stop=True,
                                    tile_position=(0, gl * 64),
                                )
                            co2 += m
                        # evac y -> fp16 slab, channel sums ride along
                        yslice = y16[h][:, b, f0:f0 + used]
                        rec = b * N_BAT + bi
                        if gidx % EV2_SCALAR_EVERY == 0:
                            nc.scalar.activation(
                                out=yslice, in_=p2[:, :used],
                                func=mybir.ActivationFunctionType.Copy,
                                accum_out=sums_t[h][:, rec:rec + 1],
                            )
                        else:
                            nc.vector.tensor_scalar(
                                out=yslice, in0=p2[:, :used],
                                scalar1=1.0, scalar2=0.0,
                                op0=mybir.AluOpType.mult,
                                op1=mybir.AluOpType.add,
                                accum_out=sums_t[h][:, rec:rec + 1],
                            )
                        gidx += 1
                    # sumsq per quarter-slice: out=(y+0)*y -> scratch,
                    # accum_out = sum(y^2)  (tensor_tensor_reduce is broken
                    # on this compiler/HW path; scalar_tensor_tensor works)
                    for s in range(4):
                        ysl = y16[h][:, b, s * 800:(s + 1) * 800]
                        nc.vector.scalar_tensor_tensor(
                            out=sqscr, in0=ysl, scalar=0.0, in1=ysl,
                            op0=mybir.AluOpType.add,
                            op1=mybir.AluOpType.mult,
                            accum_out=sq_t[h][:, b * 4 + s:b * 4 + s + 1],
                        )
                # combine + AllReduce for this half
                gsend = consts.tile([128, 2], F32, tag=f"gsend{h}",
                                    name=f"gsend{h}")
                nc.vector.tensor_reduce(
                    out=gsend[:, 0:1], in_=sums_t[h],
                    axis=mybir.AxisListType.X, op=mybir.AluOpType.add,
                )
                nc.vector.tensor_reduce(
                    out=gsend[:, 1:2], in_=sq_t[h],
                    axis=mybir.AxisListType.X, op=mybir.AluOpType.add,
                )
                nc.gpsimd.dma_start(out=cc_in[h], in_=gsend)
                nc.gpsimd.collective_compute(
                    "AllReduce",
                    mybir.AluOpType.add,
                    replica_groups=[list(range(N_CORES))],
                    ins=[cc_in[h][:, :]],
                    outs=[cc_out[h][:, :]],
                )

            def pass2_half(h):
                # ghat/delta from the AllReduce result
                gs = consts.tile([128, 2], F32, tag=f"gs{h}", name=f"gs{h}")
                nc.sync.dma_start(out=gs, in_=cc_out[h])
                mean = consts.tile([128, 1], F32, tag=f"mean{h}")
                var = consts.tile([128, 1], F32, tag=f"var{h}")
                tmp = consts.tile([128, 1], F32, tag=f"tmp{h}")
                nc.scalar.mul(out=mean, in_=gs[:, 0:1], mul=1.0 / N_PER_CH)
                nc.scalar.mul(out=var, in_=gs[:, 1:2], mul=1.0 / N_PER_CH)
                nc.vector.tensor_mul(tmp, mean, mean)
                nc.vector.tensor_sub(var, var, tmp)
                nc.scalar.activation(
                    out=var, in_=var, func=mybir.ActivationFunctionType.Sqrt,
                    bias=eps_t, scale=1.0,
                )
                nc.vector.reciprocal(out=var, in_=var)
                gh = consts.tile([128, 1], F32, tag=f"ghat{h}")
                dl = consts.tile([128, 1], F32, tag=f"delta{h}")
                nc.vector.tensor_mul(gh, gbn_t[h][:, 0:1], var)
                nc.vector.tensor_mul(tmp, mean, gh)
                nc.vector.tensor_sub(dl, gbn_t[h][:, 1:2], tmp)

                for b in range(B_LOC):
                    ysl = y16[h][:, b, :]
                    # y16 <- ghat*y16 + delta in place
                    if W4_GPSIMD:
                        nc.gpsimd.tensor_scalar(
                            out=ysl, in0=ysl,
                            scalar1=gh, scalar2=dl,
                            op0=mybir.AluOpType.mult, op1=mybir.AluOpType.add,
                        )
                    else:
                        nc.vector.tensor_scalar(
                            out=ysl, in0=ysl,
                            scalar1=gh, scalar2=dl,
                            op0=mybir.AluOpType.mult, op1=mybir.AluOpType.add,
                        )
                    xt = xr[h * B_LOC + b]
                    # x += y_bn
                    if W5_DMA_ACCUM:
                        nc.gpsimd.dma_start(out=xt, in_=ysl,
                                            accum_op=mybir.AluOpType.add)
                    else:
                        nc.vector.tensor_add(xt, xt, ysl)
                    nc.sync.dma_start(
                        out=out_d[b, h * 128:(h + 1) * 128, :], in_=xt,
                    )

            pass1_half(0)
            pass1_half(1)
            pass2_half(0)
            pass2_half(1)

    nc.finalize()
    return nc


def _prep_consts(A, A_group, conv_w, gamma, beta):
    A_sum = A.sum(axis=0)
    row_sum = np.clip(A_sum.sum(axis=-1, keepdims=True), 1e-6, None)
    A_g = (A_sum / row_sum)[None, :, :] + A_group          # (4,25,25)
    wblk = np.zeros((2, 128, 128), np.float16)
    for h in range(2):
        for gl in range(2):
            g = 2 * h + gl
            wblk[h, gl * 64:(gl + 1) * 64, gl * 64:(gl + 1) * 64] = \
                conv_w[g].T.astype(np.float16)
    eye = np.eye(5, dtype=np.float32)
    arhs = np.stack([np.kron(eye, A_g[g].T) for g in range(G)]).astype(np.float16)
    gbn = np.stack(
        [np.stack([gamma.reshape(2, 128)[h], beta.reshape(2, 128)[h]], axis=1)
         for h in range(2)]
    ).astype(np.float32)
    return wblk, np.ascontiguousarray(arhs), np.ascontiguousarray(gbn)


def _run(inputs, trace=False, **kw):
    if "nc" not in _cache:
        _cache["nc"] = _build()
    nc = _cache["nc"]
    x = np.asarray(inputs["x"], dtype=np.float32)
    wblk, arhs, gbn = _prep_consts(
        np.asarray(inputs["A"], np.float32),
        np.asarray(inputs["A_group"], np.float32),
        np.asarray(inputs["conv_w"], np.float32),
        np.asarray(inputs["gamma"], np.float32),
        np.asarray(inputs["beta"], np.float32),
    )
    xs = x.reshape(N_CORES, B_LOC, 2, 128, TW).astype(np.float16)
    in_maps = [
        {"x16": np.ascontiguousarray(xs[i]), "wblk": wblk, "arhs": arhs,
         "gbn": gbn}
        for i in range(N_CORES)
    ]
    res = run_bass_kernel_spmd(nc, in_maps, list(range(N_CORES)), trace=trace, **kw)
    out = np.concatenate([res.results[i]["out"][None] for i in range(N_CORES)])
    return out.reshape(B, C, T, V).astype(np.float32), res


def kernel(**inputs) -> np.ndarray:
    out, _ = _run(inputs)
    return out
